# revision 88
# baseline (speedup 1.0000x reference)
"""Llama GQA attention (B=1, S=2048, HID=4096, 32 Q heads / 8 KV heads, RoPE,
causal) on 8 trn2 NeuronCores, tensor-parallel over KV heads.

Per core c: q-heads 4c..4c+3, kv-head c. Device computes a partial
out_c = attn_heads_c @ Wo[:, cols_c].T ; host sums the 8 partials.

v2 layout/schedule (per core):
  - QKV weights packed into one wqkvT [HID, 768] tensor, loaded once
    (resident in SBUF); x streamed per 512-seq block, both in chunked DMAs
    so the first matmul starts ~6 us in.
  - RoPE: half-swap via two partition-offset-crossed ACT copies (no DMA),
    sin sign folded into the sin table; muls/add on DVE in bf16.
  - attention (qb outer, h inner): diagonal 128-k-chunks narrowed to their
    causal width (cols off..512); triangular 128x128 binary mask built
    on-device, applied only to the first 128 valid cols of diag chunks.
    exp on ACT (no max-sub; values are small). rowsum via ones-matmul,
    reciprocal on DVE, broadcast across partitions on GpSimd.
  - Wo interleaved with attention: after each qb's 4 heads, the 4
    s-chunks of that block run the output projection; one 2 MB fp32
    output DMA per s-chunk (16 total).
All matmuls bf16 with fp32 PSUM accumulation.
"""
import math

import numpy as np
import ml_dtypes

S = 2048
HID = 4096
D = 128
NQ = 4            # q heads per core
NCORES = 8
SB = 512          # s/q block
NSB = S // SB     # 4
NKC = S // D      # 16 k chunks
NCC = HID // D    # 32 contraction chunks
WCOLS = NQ * D + 2 * D  # 768 packed q|k|v output dims per core
SCALE = 1.0 / math.sqrt(D)
ROPE_THETA = 10000.0

BF16 = ml_dtypes.bfloat16

_CACHE = {}


def _build():
    import concourse.tile as tile
    from concourse import bacc, mybir
    from concourse.masks import make_identity, make_upper_triangular

    dt = mybir.dt
    nc = bacc.Bacc("TRN2", target_bir_lowering=False, debug=False,
                   num_devices=NCORES)

    xT = nc.dram_tensor("xT", [HID, S], dt.bfloat16, kind="ExternalInput")
    wqkvT = nc.dram_tensor("wqkvT", [HID, WCOLS], dt.bfloat16,
                           kind="ExternalInput")
    woT = nc.dram_tensor("woT", [NQ * D, HID], dt.bfloat16,
                         kind="ExternalInput")
    cosT = nc.dram_tensor("cosT", [D, S], dt.bfloat16, kind="ExternalInput")
    sinT = nc.dram_tensor("sinT", [D, S], dt.bfloat16, kind="ExternalInput")
    part = nc.dram_tensor("part", [S, HID], dt.float32, kind="ExternalOutput")

    xTr = xT.rearrange("(ko p) s -> p ko s", p=D)          # [128,32,2048]
    wr = wqkvT.rearrange("(ko p) c -> p ko c", p=D)        # [128,32,768]
    wor = woT.rearrange("(h p) e -> p h e", p=D)           # [128,4,4096]

    with tile.TileContext(nc) as tc:
        _body(nc, tc, tile, mybir, make_identity, make_upper_triangular,
              xTr, wr, wor, cosT, sinT, part)
    nc.compile()
    return nc


def _body(nc, tc, tile, mybir, make_identity, make_upper_triangular,
          xTr, wr, wor, cosT, sinT, part):
    from contextlib import ExitStack

    import concourse.bass_isa as bass_isa

    dt = mybir.dt
    AF = mybir.ActivationFunctionType
    RED_ADD = bass_isa.ReduceOp.add

    with ExitStack() as ctx:
        const = ctx.enter_context(tc.tile_pool(name="const", bufs=1))
        persist = ctx.enter_context(tc.tile_pool(name="persist", bufs=1))

        QT = persist.tile([D, NQ, S], dt.bfloat16)     # 2 MB
        KT = persist.tile([D, S], dt.bfloat16)         # 0.5 MB
        V = persist.tile([D, NKC, D], dt.bfloat16)     # 0.5 MB [s%, kc, d]
        attnT = persist.tile([D, NQ, S], dt.bfloat16)  # 2 MB

        # ---- phase 1: QKV projection + RoPE + V transpose ----
        with ExitStack() as p1:
            wpool = p1.enter_context(tc.tile_pool(name="w", bufs=1))
            xpool = p1.enter_context(tc.tile_pool(name="xp", bufs=2))
            tr = p1.enter_context(tc.tile_pool(name="tr", bufs=3))
            ps = p1.enter_context(tc.tile_pool(name="ps1", bufs=6,
                                               space="PSUM"))
            pst = p1.enter_context(tc.tile_pool(name="ps1t", bufs=1,
                                                space="PSUM"))

            wqkv = wpool.tile([D, NCC, WCOLS], dt.bfloat16)   # 6 MB resident
            xps = [xpool.tile([D, NCC, SB], dt.bfloat16, name=f"xp{i}",
                              tag="xp") for i in range(2)]
            # interleave first weight/x chunk loads so mm0 starts early
            # wqkv chunks issue from SP, x chunks from ACT: the two SEQ
            # pipelines overlap so the first matmul's inputs land sooner
            for lo, hi in ((0, 1), (1, 2), (2, 4), (4, 6), (6, 8), (8, 12),
                           (12, 16), (16, 20), (20, 24), (24, 28), (28, 32)):
                nc.sync.dma_start(out=wqkv[:, lo:hi], in_=wr[:, lo:hi])
                nc.scalar.dma_start(out=xps[0][:, lo:hi],
                                    in_=xTr[:, lo:hi, 0:SB])

            # pre-warm the PE during the startup DMA wait: the cost (and HW
            # HAM) ramp needs ~3 us of continuous activity before full
            # clock; dummy matmuls on a zeroed tile are free PE activity
            dz = const.tile([D, D], dt.bfloat16)
            nc.vector.memset(dz, 0.0)
            dps = ps.tile([D, SB], dt.float32, tag="ps", name="dps")
            for _ in range(26):
                nc.tensor.matmul(dps[:, 0:D], dz, dz, start=True, stop=True)

            cos = const.tile([D, S], dt.bfloat16)
            nc.sync.dma_start(out=cos, in_=cosT[:, :])
            sin = const.tile([D, S], dt.bfloat16)
            nc.sync.dma_start(out=sin, in_=sinT[:, :])
            ident = const.tile([D, D], dt.bfloat16)
            make_identity(nc, ident)
            tri = const.tile([D, D], dt.bfloat16)
            make_upper_triangular(nc, tri, val=1.0, diag=True)

            def rope(acc_ps, out_slice, ssl, handoff=0):
                # out = acc*cos + halfswap(acc)*sin_signfolded
                # One copy reads the psum bank, everything else works from
                # SBUF. Last block (handoff != 0): the raw copies alternate
                # ACT/DVE and the swap/muls all go to DVE, so phase-1 psum
                # drains in ~5 short copies and ACT frees for the first
                # attention exps immediately.
                raw = tr.tile([D, SB], dt.bfloat16, tag="raw")
                if handoff == 0:
                    nc.scalar.copy(out=raw, in_=acc_ps)
                elif handoff % 2 == 1:
                    nc.scalar.copy(out=raw, in_=acc_ps)
                else:
                    nc.vector.tensor_copy(out=raw, in_=acc_ps)
                sw = tr.tile([D, SB], dt.bfloat16, tag="sw")
                if handoff == 0:
                    nc.scalar.copy(out=sw[0:64, :], in_=raw[64:128, :])
                    nc.scalar.copy(out=sw[64:128, :], in_=raw[0:64, :])
                else:
                    nc.vector.tensor_copy(out=sw[0:64, :], in_=raw[64:128, :])
                    nc.vector.tensor_copy(out=sw[64:128, :], in_=raw[0:64, :])
                nc.vector.tensor_mul(out=raw, in0=raw, in1=cos[:, ssl])
                nc.vector.tensor_mul(out=sw, in0=sw, in1=sin[:, ssl])
                nc.vector.tensor_add(out=out_slice, in0=raw, in1=sw)

            def halfswap_rope(raw, dst, ssl):
                sw = tr.tile([D, SB], dt.bfloat16, tag="sw")
                nc.vector.tensor_copy(out=sw[0:64, :], in_=raw[64:128, :])
                nc.vector.tensor_copy(out=sw[64:128, :], in_=raw[0:64, :])
                nc.vector.tensor_mul(out=raw, in0=raw, in1=cos[:, ssl])
                nc.vector.tensor_mul(out=sw, in0=sw, in1=sin[:, ssl])
                nc.vector.tensor_add(out=dst, in0=raw, in1=sw)

            for sb in range(NSB - 1):
                ssl = slice(sb * SB, (sb + 1) * SB)
                xp = xps[sb % 2]
                qps = [ps.tile([D, SB], dt.float32, tag="ps",
                               name=f"qps{h}") for h in range(NQ)]
                kps = ps.tile([D, SB], dt.float32, tag="ps")
                vps = ps.tile([D, SB], dt.float32, tag="ps")
                for ci in range(NCC):
                    st, sp = (ci == 0), (ci == NCC - 1)
                    for h in range(NQ):
                        nc.tensor.matmul(qps[h],
                                         wqkv[:, ci, h * D:(h + 1) * D],
                                         xp[:, ci], start=st, stop=sp)
                    nc.tensor.matmul(kps, wqkv[:, ci, 512:640], xp[:, ci],
                                     start=st, stop=sp)
                    nc.tensor.matmul(vps, wqkv[:, ci, 640:768], xp[:, ci],
                                     start=st, stop=sp)
                    # mid-loop: prefetch next x block in 1 MB chunks
                    if ci in (7, 15, 23, 31):
                        nsl = slice((sb + 1) * SB, (sb + 2) * SB)
                        csl = slice(ci - 7, ci + 1)
                        nc.sync.dma_start(out=xps[(sb + 1) % 2][:, csl],
                                          in_=xTr[:, csl, nsl])
                # drain V first: frees a psum bank quickly for the next sb
                # and lets the PE transposes overlap the rope chain
                vsb = tr.tile([D, SB], dt.bfloat16, tag="vsb")
                nc.scalar.copy(out=vsb, in_=vps)
                for j in range(4):
                    vtp = pst.tile([D, D], dt.bfloat16, tag="vt")
                    nc.tensor.transpose(vtp, vsb[:, j * D:(j + 1) * D], ident)
                    nc.vector.tensor_copy(out=V[:, sb * 4 + j, :], in_=vtp)
                for h in range(NQ):
                    rope(qps[h], QT[:, h, ssl], ssl)
                rope(kps, KT[:, ssl], ssl)

            # ---- last seq block: two passes of 3 accumulators each, with
            # group (0,0)'s attention scores spliced into pass B so its exps
            # and the pass-A rope tails hide under projection matmuls ----
            ssl = slice((NSB - 1) * SB, NSB * SB)
            xp = xps[(NSB - 1) % 2]
            E0 = const.tile([D, 4, SB], dt.bfloat16)
            acc0 = const.tile([D, SB], dt.bfloat16)
            rs0 = const.tile([D, SB], dt.float32)
            rcpb0 = const.tile([D, SB], dt.float32)


            accA = [ps.tile([D, SB], dt.float32, tag="ps", name=f"pA{i}")
                    for i in range(3)]           # q0, q1, k
            for ci in range(NCC):
                st, sp = (ci == 0), (ci == NCC - 1)
                nc.tensor.matmul(accA[0], wqkv[:, ci, 0:D], xp[:, ci],
                                 start=st, stop=sp)
                nc.tensor.matmul(accA[1], wqkv[:, ci, D:2 * D], xp[:, ci],
                                 start=st, stop=sp)
                nc.tensor.matmul(accA[2], wqkv[:, ci, 512:640], xp[:, ci],
                                 start=st, stop=sp)
            rawsA = []
            for idx, acc in enumerate(accA):
                raw = tr.tile([D, SB], dt.bfloat16, tag="rawL", bufs=5,
                              name=f"rwA{idx}")
                if idx % 2 == 0:
                    nc.scalar.copy(out=raw, in_=acc)
                else:
                    nc.vector.tensor_copy(out=raw, in_=acc)
                rawsA.append(raw)
            for idx, raw in enumerate(rawsA):
                dst = QT[:, idx, ssl] if idx < 2 else KT[:, ssl]
                halfswap_rope(raw, dst, ssl)

            accB = [ps.tile([D, SB], dt.float32, tag="ps", name=f"pB{i}")
                    for i in range(3)]           # q2, q3, v
            for ci in range(NCC):
                st, sp = (ci == 0), (ci == NCC - 1)
                nc.tensor.matmul(accB[0], wqkv[:, ci, 2 * D:3 * D],
                                 xp[:, ci], start=st, stop=sp)
                nc.tensor.matmul(accB[1], wqkv[:, ci, 3 * D:4 * D],
                                 xp[:, ci], start=st, stop=sp)
                nc.tensor.matmul(accB[2], wqkv[:, ci, 640:768], xp[:, ci],
                                 start=st, stop=sp)
                if ci in (2, 9, 16, 23):         # splice group (0,0) scores
                    j = (2, 9, 16, 23).index(ci)
                    off = j * D
                    w = SB - off
                    stp = ps.tile([D, SB], dt.float32, tag="st0", bufs=1,
                                  name=f"st0{j}")
                    nc.tensor.matmul(stp[:, 0:w], KT[:, j * D:(j + 1) * D],
                                     QT[:, 0, off:SB], start=True, stop=True)
                    nc.scalar.activation(out=E0[:, j, off:SB],
                                         in_=stp[:, 0:w],
                                         func=AF.Exp, scale=SCALE)
                    nc.vector.tensor_mul(out=E0[:, j, off:off + D],
                                         in0=E0[:, j, off:off + D], in1=tri)
                    if j == 0:
                        nc.vector.tensor_copy(out=acc0, in_=E0[:, 0, :])
                    else:
                        nc.vector.tensor_add(out=acc0[:, off:SB],
                                             in0=acc0[:, off:SB],
                                             in1=E0[:, j, off:SB])
            nc.gpsimd.partition_all_reduce(rs0, acc0, channels=D,
                                           reduce_op=RED_ADD)
            nc.vector.reciprocal(out=rcpb0, in_=rs0)
            vsb = tr.tile([D, SB], dt.bfloat16, tag="vsb")
            nc.scalar.copy(out=vsb, in_=accB[2])
            for j in range(4):
                vtp = pst.tile([D, D], dt.bfloat16, tag="vt")
                nc.tensor.transpose(vtp, vsb[:, j * D:(j + 1) * D], ident)
                nc.vector.tensor_copy(out=V[:, (NSB - 1) * 4 + j, :],
                                      in_=vtp)
            rawsB = []
            for idx in range(2):
                raw = tr.tile([D, SB], dt.bfloat16, tag="rawL", bufs=5,
                              name=f"rwB{idx}")
                if idx % 2 == 0:
                    nc.scalar.copy(out=raw, in_=accB[idx])
                else:
                    nc.vector.tensor_copy(out=raw, in_=accB[idx])
                rawsB.append(raw)
            for idx, raw in enumerate(rawsB):
                halfswap_rope(raw, QT[:, 2 + idx, ssl], ssl)

        # ---- phase 2+3 interleaved: attention + output projection ----
        with ExitStack() as p2:
            epool = p2.enter_context(tc.tile_pool(name="E", bufs=3))
            wopool = p2.enter_context(tc.tile_pool(name="wo", bufs=1))
            outp = p2.enter_context(tc.tile_pool(name="outp", bufs=3))
            tr2 = p2.enter_context(tc.tile_pool(name="tr2", bufs=3))
            ps_s = p2.enter_context(tc.tile_pool(name="ps_s", bufs=3,
                                                 space="PSUM"))
            ps_pv = p2.enter_context(tc.tile_pool(name="ps_pv", bufs=2,
                                                  space="PSUM"))
            ps_o = p2.enter_context(tc.tile_pool(name="ps_o", bufs=3,
                                                 space="PSUM"))

            woSb = wopool.tile([D, NQ, HID], dt.bfloat16)  # 4 MB
            nc.sync.dma_start(out=woSb, in_=wor)

            # software pipeline: group g's scores/exp emit interleaved with
            # group g-1's PV so ACT always runs one group ahead of the PE's
            # PV consumption. The softmax denominator is built off the PE:
            # DVE accumulates the exp chunks elementwise, one GpSimd
            # partition_all_reduce sums over k (output broadcast to all
            # partitions for free). Completed blocks' output projections are
            # queued as per-eb tasks and paced one per chunk iteration so
            # the PE stays fed while ACT digests the exp backlog.
            groups = [(qb, h) for qb in range(NSB) for h in range(NQ)]

            def chunk_list(qb):
                # full-width diag chunk first (opens the psum group), then
                # narrow diag chunks spread among full off-diag ones to keep
                # the PE-work-per-exp ratio even across the group
                diag = [(4 * qb + j, j * D) for j in range(1, 4)]
                off = [(kc, 0) for kc in range(4 * qb)]
                rest = []
                while diag or off:
                    if off:
                        rest.append(off.pop(0))
                    if diag:
                        rest.append(diag.pop(0))
                return [(4 * qb, 0)] + rest

            wo_fifo = []
            osb_live = [None]          # current [D, HID] staging tile

            def emit_wo_task():
                sc, eb = wo_fifo.pop(0)
                last_sc = (sc == S // D - 1)
                scl = slice(sc * D, (sc + 1) * D)
                esl = slice(eb * SB, (eb + 1) * SB)
                if eb == 0:
                    # all row blocks stage in bf16: halves every output DMA
                    # and the staging SBUF (so 5 buffers fit); the host
                    # decodes the raw bf16 from the fp32 part region
                    osb_live[0] = outp.tile([D, HID], dt.bfloat16,
                                            tag="o", bufs=5, name="osb")
                osb = osb_live[0]
                op = ps_o.tile([D, SB], dt.float32, tag="op", name="op")
                for h in range(NQ):
                    nc.tensor.matmul(op, attnT[:, h, scl], woSb[:, h, esl],
                                     start=(h == 0), stop=(h == NQ - 1))
                if eb % 2 == 0:
                    nc.scalar.copy(out=osb[:, esl], in_=op)
                else:
                    nc.vector.tensor_copy(out=osb[:, esl], in_=op)
                # write out incrementally so the transfer overlaps the later
                # eb matmuls; the very last row block goes in quarters to
                # minimize the end-of-kernel drain
                if sc >= S // D - 4:
                    # final blocks: progressively smaller pieces, issued
                    # early, so the trailing transfers don't bunch up on
                    # the shared DMA queue after the last matmul
                    if eb in (1, 3, 5):
                        esl2 = slice((eb - 1) * SB, (eb + 1) * SB)
                        nc.sync.dma_start(
                            out=part[scl,
                                     (eb - 1) * SB // 2:(eb + 1) * SB // 2],
                            in_=osb[:, esl2].bitcast(dt.float32))
                    elif eb >= 6:
                        nc.sync.dma_start(
                            out=part[scl, eb * SB // 2:(eb + 1) * SB // 2],
                            in_=osb[:, esl].bitcast(dt.float32))
                elif eb == HID // SB // 2 - 1:
                    nc.sync.dma_start(
                        out=part[scl, 0:HID // 4],
                        in_=osb[:, 0:HID // 2].bitcast(dt.float32))
                elif eb == HID // SB - 1:
                    nc.sync.dma_start(
                        out=part[scl, HID // 4:HID // 2],
                        in_=osb[:, HID // 2:].bitcast(dt.float32))

            # group (0,0) was pre-computed (scores/exp/denominator) inside
            # the last projection block; the pipeline starts with it as prv
            E_prev, chunks_prev, rcpb_prev = E0, chunk_list(0), rcpb0
            for g in range(1, len(groups) + 1):
                cur = groups[g] if g < len(groups) else None
                prv = groups[g - 1] if g >= 1 else None
                if cur is not None:
                    qbc, hc = cur
                    qs0c = qbc * SB
                    chunks_cur = chunk_list(qbc)
                    E_cur = epool.tile([D, NKC, SB], dt.bfloat16, tag="E")
                    acc_cur = tr2.tile([D, SB], dt.bfloat16, tag="acc")
                if prv is not None:
                    qbp, hp = prv
                    qs0p = qbp * SB
                    pvp = ps_pv.tile([D, SB], dt.float32, tag="pv")
                    npv = len(chunks_prev)
                    rcpb = rcpb_prev  # issued at the end of prv's iteration
                nsc = len(chunks_cur) if cur is not None else 0
                for i in range(max(nsc, npv if prv is not None else 0)):
                    if cur is not None and i < nsc:
                        kc, off = chunks_cur[i]
                        w = SB - off
                        stp = ps_s.tile([D, SB], dt.float32, tag="s")
                        nc.tensor.matmul(stp[:, 0:w],
                                         KT[:, kc * D:(kc + 1) * D],
                                         QT[:, hc, qs0c + off:qs0c + SB],
                                         start=True, stop=True)
                        nc.scalar.activation(out=E_cur[:, kc, off:SB],
                                             in_=stp[:, 0:w],
                                             func=AF.Exp, scale=SCALE)
                        if off or kc == 4 * qbc:  # diag chunk: tri edge
                            # warmup groups: DVE is backlogged with the last
                            # block's rope tails, so use the idle GpSimd
                            eng = nc.gpsimd if g < 6 else nc.vector
                            eng.tensor_mul(
                                out=E_cur[:, kc, off:off + D],
                                in0=E_cur[:, kc, off:off + D], in1=tri)
                        if i == 0:
                            nc.vector.tensor_copy(out=acc_cur,
                                                  in_=E_cur[:, kc, :])
                        else:
                            nc.vector.tensor_add(
                                out=acc_cur[:, off:SB],
                                in0=acc_cur[:, off:SB],
                                in1=E_cur[:, kc, off:SB])
                    if prv is not None and i < npv:
                        kc, off = chunks_prev[i]
                        st = (i == 0)
                        sp = (i == 0) if qbp == 0 else (i == npv - 1)
                        skip = (qbp == 0 and i > 0)
                        nc.tensor.matmul(pvp[:, off:SB], V[:, kc],
                                         E_prev[:, kc, off:SB],
                                         start=st, stop=sp,
                                         skip_group_check=skip)
                        if i == npv - 1:
                            # emit the norm right behind the last PV so the
                            # psum bank frees before DVE's remaining backlog
                            nc.vector.tensor_mul(
                                out=attnT[:, hp, qs0p:qs0p + SB],
                                in0=pvp, in1=rcpb)
                    # keep ~8 tasks in reserve: they carry no dependency on
                    # the newest norm, so they fill the PE while each
                    # denominator chain (PAR -> recip -> norm) completes.
                    # Score-only tail iterations always take filler.
                    if wo_fifo and (len(wo_fifo) > 4 or g == len(groups)
                                    or prv is None or i >= npv):
                        emit_wo_task()
                if prv is not None:
                    if hp == NQ - 1:
                        for sc in range(4 * qbp, 4 * qbp + 4):
                            for eb in range(HID // SB):
                                wo_fifo.append((sc, eb))
                if cur is not None:
                    # acc_cur is complete: start the denominator now so the
                    # reciprocal is long done when cur's PV finishes next
                    # iteration (only the norm mul stays on the chain)
                    rs = tr2.tile([D, SB], dt.float32, tag="rsb")
                    nc.gpsimd.partition_all_reduce(rs, acc_cur, channels=D,
                                                   reduce_op=RED_ADD)
                    rcpb_cur = tr2.tile([D, SB], dt.float32, tag="rcpb")
                    nc.vector.reciprocal(out=rcpb_cur, in_=rs)
                    E_prev, chunks_prev = E_cur, chunks_cur
                    rcpb_prev = rcpb_cur
            while wo_fifo:  # trailing output projections for the last block
                emit_wo_task()


def _prep(hidden_states, attention_mask, position_ids, Wq, Wk, Wv, Wo):
    """Host-side sharding/layout. Returns per-core input maps."""
    x = np.asarray(hidden_states, dtype=np.float32)[0]          # [S, HID]
    xT = np.ascontiguousarray(x.T).astype(BF16)                 # [HID, S]

    pos = np.asarray(position_ids)[0].astype(np.float64)        # [S]
    inv = 1.0 / (ROPE_THETA ** (np.arange(0, D, 2, dtype=np.float64) / D))
    ang = np.empty((D, S), dtype=np.float64)
    ang[:64] = inv[:, None] * pos[None, :]
    ang[64:] = ang[:64]
    cosT = np.cos(ang).astype(BF16)
    sinT = np.sin(ang)
    sinT[:64] *= -1.0                                           # sign folded
    sinT = sinT.astype(BF16)

    Wq = np.asarray(Wq, dtype=np.float32)
    Wk = np.asarray(Wk, dtype=np.float32)
    Wv = np.asarray(Wv, dtype=np.float32)
    Wo = np.asarray(Wo, dtype=np.float32)

    in_maps = []
    for c in range(NCORES):
        qsl = slice(c * NQ * D, (c + 1) * NQ * D)
        ksl = slice(c * D, (c + 1) * D)
        wqkv = np.concatenate([Wq[qsl, :], Wk[ksl, :], Wv[ksl, :]], axis=0)
        in_maps.append({
            "xT": xT,
            "wqkvT": np.ascontiguousarray(wqkv.T).astype(BF16),
            "woT": np.ascontiguousarray(Wo[:, qsl].T).astype(BF16),
            "cosT": cosT, "sinT": sinT,
        })
    return in_maps


def kernel(hidden_states, attention_mask, position_ids, Wq, Wk, Wv, Wo,
           _trace=False):
    from concourse.bass_utils import run_bass_kernel_spmd

    if "nc" not in _CACHE:
        _CACHE["nc"] = _build()
    nc = _CACHE["nc"]

    in_maps = _prep(hidden_states, attention_mask, position_ids, Wq, Wk, Wv, Wo)
    res = run_bass_kernel_spmd(nc, in_maps, core_ids=list(range(NCORES)),
                               trace=_trace)
    _CACHE["last_res"] = res
    out = np.zeros((S, HID), dtype=np.float64)
    for c in range(NCORES):
        p = np.ascontiguousarray(res.results[c]["part"])
        # every row block is raw bf16 packed into the front fp32 columns
        out += p[:, :HID // 2].view(BF16).astype(np.float64)
    return out.astype(np.float32).reshape(1, S, HID)


if __name__ == "__main__":
    pass
